# revision 32
# baseline (speedup 1.0000x reference)
"""ChatGLM2 attention block on 8 Trainium2 NeuronCores (Bass/Tile).

Sharding: tensor-parallel across heads. Core c owns Q heads 4c..4c+3
(512 dims, projected in fp8 DoubleRow at 2x PE rate — sim-verified to
leave the output metric unchanged since Q/K errors wash out in softmax);
K/V projection (64-col [K32|V32] rank slice, fp16 for V precision) is
rebuilt per batch with an intra-group AllGather. Dense is sharded 2x4
(token-half x output-quarter) behind a per-batch AllToAll.

Schedule: batch-0 K/V is computed locally in full (each core does its
group's 256 K+V dims) so attention(0) needs no collective and the first
collective (~70us startup-barrier floor) is batch-1's small AllGather,
issued before AllToAll(0) on the serial CC stream. KV blocks run before
Q blocks (fp8 x/w stream in behind the fp16 one), attention(0) sits
between Q blocks 1 and 2 so AllToAll(0) flies during Q blocks 2-3, and
AllToAll(1) rides behind dense(batch 0). All x/w DMAs use host-side
contiguous tile layouts split across the sync and scalar queues. The PE
is GPIO power-throttled to ~81% duty with 8 cores active (~1.95 cols/ns
fp16 sustained), which sets the compute floor.
"""

import math
import sys
import types

import numpy as np

# ---------------------------------------------------------------- constants
B, S, H = 2, 1024, 4096
NH, G, HD = 32, 2, 128
ROT = 64
N_CORES = 8
TOK = B * S                      # 2048
HPC = NH // N_CORES              # 4 Q heads per core
DPC = HPC * HD                   # 512 Q dims per core
TB = 4                           # token blocks of 512
QB = 2                           # q blocks of 512 per batch
ODPC = H // 4                    # 1024 output dims per core (dense quarter)
SCALE = 1.0 / math.sqrt(HD)
X8S = 32.0                       # fp8 scale for hidden states
W8S = 32.0                       # fp8 scale for Q weights
QDESC = 1.0 / (X8S * W8S)


def _install_ntff_hook():
    if "antenv.axon_hooks" in sys.modules:
        return
    import antenv  # noqa: F401

    mod = types.ModuleType("antenv.axon_hooks")
    mod._hook = None
    mod.set_axon_ntff_profile_hook = lambda h: setattr(mod, "_hook", h)
    mod.get_axon_ntff_profile_hook = lambda: mod._hook
    sys.modules["antenv.axon_hooks"] = mod
    try:
        from trn_agent_boot.trn_boot import _ntff_profile_via_ctypes

        mod._hook = _ntff_profile_via_ctypes("/opt/axon/libaxon_pjrt.so")
    except Exception:
        pass


_install_ntff_hook()

import concourse.bass as bass  # noqa: E402
import concourse.mybir as mybir  # noqa: E402
import concourse.tile as tile  # noqa: E402
from concourse import bacc  # noqa: E402
from concourse.bass_utils import run_bass_kernel_spmd  # noqa: E402

F32 = mybir.dt.float32
F16 = mybir.dt.float16
F8 = mybir.dt.float8e4
AF = mybir.ActivationFunctionType
ALU = mybir.AluOpType
PM = mybir.MatmulPerfMode


# ---------------------------------------------------------------- build
def build(trace_label="k"):
    nc = bacc.Bacc("TRN2", target_bir_lowering=False, debug=False,
                   num_devices=N_CORES)

    xt_d = nc.dram_tensor("xt", [16, 128, 4096], F16,
                          kind="ExternalInput").ap()
    x8_d = nc.dram_tensor("x8", [4, 128, 16384], F8,
                          kind="ExternalInput").ap()
    wkv_d = nc.dram_tensor("wkv", [128, 2048], F16,
                           kind="ExternalInput").ap()
    wkvf_d = nc.dram_tensor("wkvf", [128, 8192], F16,
                            kind="ExternalInput").ap()
    wq8_d = nc.dram_tensor("wq8", [128, 16384], F8,
                           kind="ExternalInput").ap()
    bq_d = nc.dram_tensor("bqkv", [128, 7], F32, kind="ExternalInput").ap()
    # packed f16 consts: [ ones(128) | tri(128) | ident(128) | perm(64c) ]
    cc_d = nc.dram_tensor("consts", [128, 448], F16, kind="ExternalInput").ap()
    rq_d = nc.dram_tensor("ropeQ", [128, TOK], F16, kind="ExternalInput").ap()
    rk_d = nc.dram_tensor("ropeK", [64, TOK], F16, kind="ExternalInput").ap()
    wd_d = nc.dram_tensor("wd", [4, 128, 8192], F16,
                          kind="ExternalInput").ap()
    out_d = nc.dram_tensor("out", [ODPC, 1024], F32, kind="ExternalOutput").ap()

    from contextlib import ExitStack

    with tile.TileContext(nc) as tc:
        with tc.tile_pool(name="consts", bufs=1) as cp, \
             tc.tile_pool(name="dram", bufs=1, space="DRAM") as dp:
            # ---- small constants (alive whole kernel)
            bias_sb = cp.tile([128, 7], F32, tag="bias")
            cc_sb = cp.tile([128, 448], F16, tag="consts")
            nc.sync.dma_start(bias_sb[:], bq_d[:])
            nc.sync.dma_start(cc_sb[:], cc_d[:])
            oc_sb = cc_sb[:, 0:128]
            tri_sb = cc_sb[:, 128:256]
            id_sb = cc_sb[:, 256:384]
            perm_sb = cc_sb[0:64, 384:448]
            # rope planes for all 4 blocks (loads emitted after x16(0))
            rqall = cp.tile([128, TOK], F16, tag="ropeQ")
            rkall = cp.tile([64, TOK], F16, tag="ropeK")

            def load_rope():
                nc.sync.dma_start(rqall[:], rq_d[:])
                nc.sync.dma_start(rkall[:], rk_d[:])

            # DRAM staging for collectives
            kv_loc = [dp.tile([64, 1024], F16, tag=f"kvl{b}",
                              name=f"kvl{b}") for b in range(B)]
            kvg = [dp.tile([4, 64, 1024], F16, tag=f"kvg{b}",
                           name=f"kvg{b}") for b in range(B)]
            a2a_in = [dp.tile([8, 512, 512], F16, tag=f"a2i{b}",
                              name=f"a2i{b}") for b in range(B)]
            a2a_out = [dp.tile([8, 512, 512], F16, tag=f"a2o{b}",
                               name=f"a2o{b}") for b in range(B)]
            a2ah_in = [dp.tile([8, 256, 512], F16, tag=f"a2hi{h_}",
                               name=f"a2hi{h_}") for h_ in range(2)]
            a2ah_out = [dp.tile([8, 256, 512], F16, tag=f"a2ho{h_}",
                                name=f"a2ho{h_}") for h_ in range(2)]

            es = ExitStack()
            pp = es.enter_context(
                tc.tile_pool(name="ps_main", bufs=8, space="PSUM"))
            kp = es.enter_context(tc.tile_pool(name="kvp", bufs=1))
            qtp = es.enter_context(tc.tile_pool(name="qtp", bufs=1))
            xcp = es.enter_context(tc.tile_pool(name="ctxp", bufs=5))
            ep = es.enter_context(tc.tile_pool(name="exp", bufs=8))
            sp = es.enter_context(tc.tile_pool(name="att_small", bufs=2))
            es2 = ExitStack()
            wp = es2.enter_context(tc.tile_pool(name="wq", bufs=1))
            xp = es2.enter_context(tc.tile_pool(name="xt", bufs=8))
            x8p = es2.enter_context(tc.tile_pool(name="x8", bufs=2))
            rp = es2.enter_context(tc.tile_pool(name="rope_tmp", bufs=2))
            kqp = es2.enter_context(tc.tile_pool(name="kq", bufs=2))

            # per-batch K / V(transposed) tiles
            ktile = [kp.tile([128, 1024], F16, tag=f"k{b}", name=f"k{b}")
                     for b in range(B)]
            vtile = [kp.tile([128, 1024], F16, tag=f"vt{b}", name=f"vt{b}")
                     for b in range(B)]
            vn = [kp.tile([128, 1024], F16, tag=f"vn{b}", name=f"vn{b}")
                  for b in range(B)]
            qtl = {}

            # ---- weights (scalar queue: parallel to the x16 sync stream)
            wkvf_sb = wp.tile([128, 32, 256], F16, tag="wkvf", name="wkvf")
            nc.scalar.dma_start(wkvf_sb[:].rearrange("p a b -> p (a b)"),
                                wkvf_d[:])
            wkv_sb = wp.tile([128, 32, 64], F16, tag="wkv", name="wkv")
            nc.scalar.dma_start(wkv_sb[:].rearrange("p a b -> p (a b)"),
                                wkv_d[:])
            wq8_sb = wp.tile([128, 32, DPC], F8, tag="wq8", name="wq8")

            def load_wq8():
                nc.scalar.dma_start(wq8_sb[:].rearrange("p a b -> p (a b)"),
                                    wq8_d[:])
            xg16_pre = {}
            xg8_pre = {}

            def prefetch_x16(t):
                for gg in range(4):
                    xg = xp.tile([128, 8, 512], F16, tag="xtblk")
                    nc.sync.dma_start(xg[:].rearrange("p a b -> p (a b)"),
                                      xt_d[t * 4 + gg])
                    xg16_pre[(t, gg)] = xg

            def prefetch_x8(t):
                x8t = x8p.tile([128, 32, 512], F8, tag="x8blk")
                nc.scalar.dma_start(x8t[:].rearrange("p a b -> p (a b)"),
                                    x8_d[t])
                xg8_pre[t] = x8t

            def rope64(dst, qab):
                """In-place ChatGLM2 rotary on rows 0:64 of a [128,512]
                f16 tile (rot dims 0:64, pass dims 64:128)."""
                sw = pp.tile([128, 512], F32, tag="bank", name="swps")
                nc.tensor.matmul(sw[0:ROT, :], perm_sb[:],
                                 dst[0:ROT, :], start=True, stop=True)
                t1 = rp.tile([ROT, 512], F32, tag="t1")
                nc.vector.tensor_mul(t1[:], dst[0:ROT, :], qab[0:64, :])
                t2 = rp.tile([ROT, 512], F32, tag="t2")
                nc.vector.tensor_mul(t2[:], sw[0:ROT, :], qab[64:128, :])
                nc.vector.tensor_add(dst[0:ROT, :], t1[:], t2[:])

            def kv_block_full(t):
                """Batch-0 blocks: full group K[128]+V[128] computed locally
                (no AllGather on the critical path for attention(0))."""
                half = t % 2
                xgs = [xg16_pre.pop((t, gg)) for gg in range(4)]
                kps = pp.tile([128, 512], F32, tag="bank", name="kfps")
                vps = pp.tile([128, 512], F32, tag="bank", name="vfps")
                for gg in range(4):
                    for kk in range(8):
                        k = gg * 8 + kk
                        nc.tensor.matmul(kps[:], wkvf_sb[:, k, 0:128],
                                         xgs[gg][:, kk, :],
                                         start=(k == 0), stop=(k == 31))
                ksl = ktile[0][:, half * 512:(half + 1) * 512]
                nc.scalar.activation(ksl, kps[:], AF.Identity,
                                     bias=bias_sb[:, 5:6])
                for gg in range(4):
                    for kk in range(8):
                        k = gg * 8 + kk
                        nc.tensor.matmul(vps[:], wkvf_sb[:, k, 128:256],
                                         xgs[gg][:, kk, :],
                                         start=(k == 0), stop=(k == 31))
                nc.scalar.activation(vtile[0][:, half * 512:(half + 1) * 512],
                                     vps[:], AF.Identity,
                                     bias=bias_sb[:, 6:7])
                rope64(ksl, rqall[:, t * 512:(t + 1) * 512])

            def kv_block_slice(t):
                """Batch-1 blocks: 64-col [K32|V32] rank slice -> kv_loc."""
                b, half = t // 2, t % 2
                xgs = [xg16_pre.pop((t, gg)) for gg in range(4)]
                ps_kv = pp.tile([128, 512], F32, tag="bank", name="kvps")
                for gg in range(4):
                    for kk in range(8):
                        k = gg * 8 + kk
                        nc.tensor.matmul(
                            ps_kv[0:64, :],
                            wkv_sb[:, k, :],
                            xgs[gg][:, kk, :],
                            start=(k == 0), stop=(k == 31),
                        )
                kq = kqp.tile([64, 512], F16, tag="kvtile", name=f"kv{t}")
                nc.scalar.activation(kq[:], ps_kv[0:64, :], AF.Identity,
                                     bias=bias_sb[0:64, 4:5])
                kab = rkall[:, t * 512:(t + 1) * 512]
                # K slice rope (identity planes on ranks holding pass-dims)
                swk = pp.tile([128, 512], F32, tag="bank", name="swkps")
                nc.tensor.matmul(swk[0:32, :], perm_sb[0:32, 0:32],
                                 kq[0:32, :], start=True, stop=True)
                t1 = rp.tile([32, 512], F32, tag="t1k", bufs=1)
                nc.vector.tensor_mul(t1[:], kq[0:32, :], kab[0:32, :])
                t2 = rp.tile([32, 512], F32, tag="t2k", bufs=1)
                nc.vector.tensor_mul(t2[:], swk[0:32, :], kab[32:64, :])
                nc.vector.tensor_add(kq[0:32, :], t1[:], t2[:])
                nc.sync.dma_start(
                    kv_loc[b][:, half * 512:(half + 1) * 512], kq[:])

            def q_block(t):
                """Q projection (fp8 DoubleRow, 4 psum banks) + bias + RoPE."""
                x8t = xg8_pre.pop(t)
                ps = [pp.tile([128, 512], F32, tag="bank",
                              name=f"qps{d}") for d in range(HPC)]
                for kp2 in range(16):
                    for d in range(HPC):
                        nc.tensor.matmul(
                            ps[d][:],
                            wq8_sb[:, 2 * kp2:2 * kp2 + 2,
                                   d * 128:(d + 1) * 128],
                            x8t[:, 2 * kp2:2 * kp2 + 2, :],
                            start=(kp2 == 0), stop=(kp2 == 15),
                            perf_mode=PM.DoubleRow,
                        )
                for h in range(HPC):
                    qt = qtp.tile([128, 512], F16,
                                  tag=f"q{h}_{t}", name=f"q{h}_{t}")
                    qtl[(h, t)] = qt
                    nc.scalar.activation(qt[:], ps[h][:], AF.Identity,
                                         bias=bias_sb[:, h:h + 1],
                                         scale=QDESC)
                for e in range(HPC):
                    rope64(qtl[(e, t)], rqall[:, t * 512:(t + 1) * 512])

            def kv_allgather(b):
                nc.gpsimd.collective_compute(
                    "AllGather", ALU.bypass,
                    replica_groups=[[0, 1, 2, 3], [4, 5, 6, 7]],
                    ins=[kv_loc[b][:].opt()],
                    outs=[kvg[b][:].opt()])

            def assemble_loads(b):
                """ktile/vtile from the gathered per-rank 64-col slices."""
                for r in range(4):
                    nc.gpsimd.dma_start(ktile[b][32 * r:32 * r + 32, :],
                                        kvg[b][r, 0:32, :])
                    nc.gpsimd.dma_start(vtile[b][32 * r:32 * r + 32, :],
                                        kvg[b][r, 32:64, :])

            def build_vn(b):
                for jj in range(2):
                    tp = pp.tile([128, 512], F16, tag="bank", name="vtrps")
                    for j in range(4):
                        nc.tensor.transpose(
                            tp[:, j * 128:(j + 1) * 128],
                            vtile[b][:, (jj * 4 + j) * 128:
                                     (jj * 4 + j + 1) * 128],
                            id_sb[:])
                    nc.scalar.copy(vn[b][:, jj * 512:(jj + 1) * 512], tp[:])

            def attn_batch(b):
                """Software-pipelined attention for one batch: flattened
                (qb, h, kt) stream with lookahead-2 sc -> rs/av."""
                units = [(qb, h) for qb in range(QB) for h in range(HPC)]
                tasks = []
                for u, (qb, h) in enumerate(units):
                    for kt in range(4 * (qb + 1)):
                        tasks.append((u, kt))
                n_kt = {u: 4 * (units[u][0] + 1) for u in range(len(units))}
                rs_ps, ctx_ps = {}, {}

                def emit_sc(u, kt):
                    qb, h = units[u]
                    tb = b * QB + qb
                    off = max(0, (kt - qb * 4) * 128)
                    N = 512 - off
                    sc = pp.tile([128, 512], F32, tag="bank", name="scps")
                    nc.tensor.matmul(sc[:, 0:N],
                                     ktile[b][:, kt * 128:(kt + 1) * 128],
                                     qtl[(h, tb)][:, off:512],
                                     start=True, stop=True)
                    e = ep.tile([128, 512], F16, tag="exp")
                    nc.scalar.activation(e[:, 0:N], sc[:, 0:N],
                                         AF.Exp, scale=SCALE)
                    if kt >= qb * 4:  # diagonal straddle: first 128 cols
                        nc.vector.tensor_mul(e[:, 0:128], e[:, 0:128],
                                             tri_sb[:])
                    return (e, off, N)

                def emit_rsav(u, kt, e, off, N):
                    qb, h = units[u]
                    if kt == 0:
                        rs_ps[u] = pp.tile([128, 512], F32, tag="bank",
                                           name="rsps")
                        ctx_ps[u] = pp.tile([128, 512], F32, tag="bank",
                                            name="ctxps")
                    first, last = kt == 0, kt == n_kt[u] - 1
                    nc.tensor.matmul(rs_ps[u][:, off:512], oc_sb[:],
                                     e[:, 0:N], start=first, stop=last)
                    nc.tensor.matmul(ctx_ps[u][:, off:512],
                                     vn[b][:, kt * 128:(kt + 1) * 128],
                                     e[:, 0:N], start=first, stop=last)
                    if last:
                        rcp = sp.tile([128, 512], F32, tag="rcp")
                        nc.vector.reciprocal_approx_fast(
                            out=rcp[:], in_=rs_ps[u][:])
                        ctxt = xcp.tile([128, 512], F16, tag="ctx")
                        nc.vector.tensor_mul(ctxt[:], ctx_ps[u][:], rcp[:])
                        # one DMA writes all 4 dup blocks: src broadcasts
                        # via a stride-0 dim after the partition dim
                        cap = ctxt[:]
                        bsrc = bass.AP(cap.tensor, cap.offset,
                                       [cap.ap[0], (0, 4), cap.ap[1]])
                        dst = a2a_in[b].rearrange("j p t -> p j t")[
                            h * 128:(h + 1) * 128, qb * 4:qb * 4 + 4, :]
                        nc.scalar.dma_start(dst, bsrc)

                pend = {}
                for i, (u, kt) in enumerate(tasks):
                    pend[i] = (u, kt) + emit_sc(u, kt)
                    if i - 4 >= 0:
                        emit_rsav(*pend.pop(i - 4))
                for j in sorted(pend):
                    emit_rsav(*pend.pop(j))

            def a2a(b):
                nc.gpsimd.collective_compute(
                    "AllToAll", ALU.bypass,
                    replica_groups=[list(range(N_CORES))],
                    ins=[a2a_in[b][:].opt()],
                    outs=[a2a_out[b][:].opt()])

            def a2a_half(h_):
                nc.gpsimd.collective_compute(
                    "AllToAll", ALU.bypass,
                    replica_groups=[list(range(N_CORES))],
                    ins=[a2ah_in[h_][:].opt()],
                    outs=[a2ah_out[h_][:].opt()])

            wd_sb = []

            def load_wd_all(wdp):
                for g in range(4):
                    wg = wdp.tile([128, 8, ODPC], F16, tag=f"wd{g}")
                    nc.sync.dma_start(wg[:].rearrange("p a b -> p (a b)"),
                                      wd_d[g])
                    wd_sb.append(wg)

            def load_cg_chunks(cgp, half):
                """Per-group cg tiles: dense can start on chunk 0 ~3us
                after the AllToAll lands instead of waiting a 4MB load."""
                cgs = a2a_out[half].rearrange("s (sub p) t -> p (s sub) t",
                                              sub=4)
                out = []
                for g in range(4):
                    cgt = cgp.tile([128, 8, 512], F16, tag=f"cg{half}{g}")
                    nc.sync.dma_start(cgt[:], cgs[:, g * 8:(g + 1) * 8, :])
                    out.append(cgt)
                return out

            def dense_half2(op_, cgch, half):
                ps = [pp.tile([128, 512], F32, tag="bank",
                              name=f"dps{half}_{odb}") for odb in range(8)]
                for g in range(4):
                    for odb in range(8):
                        for kk in range(8):
                            nc.tensor.matmul(
                                ps[odb][:],
                                wd_sb[g][:, kk, odb * 128:(odb + 1) * 128],
                                cgch[g][:, kk, :],
                                start=(g == 0 and kk == 0),
                                stop=(g == 3 and kk == 7))
                for odb in range(8):
                    o = op_.tile([128, 512], F32, tag="osb")
                    nc.scalar.copy(o[:], ps[odb][:])
                    nc.scalar.dma_start(
                        out_d[odb * 128:(odb + 1) * 128,
                              half * 512:(half + 1) * 512], o[:])

            # ---------------- schedule ----------------
            prefetch_x16(0)
            prefetch_x16(1)
            load_rope()
            prefetch_x8(0)
            prefetch_x8(1)
            load_wq8()
            kv_block_full(0)
            prefetch_x16(2)
            kv_block_full(1)
            prefetch_x16(3)
            build_vn(0)
            kv_block_slice(2)
            kv_block_slice(3)
            kv_allgather(1)
            assemble_loads(1)
            q_block(0)
            prefetch_x8(2)
            q_block(1)
            prefetch_x8(3)
            attn_batch(0)
            a2a(0)
            q_block(2)
            q_block(3)
            build_vn(1)
            es2.close()

            # dense-side pools come alive only after the proj pools free
            wdp = es.enter_context(
                tc.tile_pool(name="wd", bufs=1, side="right"))
            cgp = es.enter_context(
                tc.tile_pool(name="cg", bufs=1, side="right"))
            op_ = es.enter_context(
                tc.tile_pool(name="dout", bufs=4, side="right"))
            load_wd_all(wdp)
            cg0 = load_cg_chunks(cgp, 0)
            attn_batch(1)
            a2a(1)
            cg1 = load_cg_chunks(cgp, 1)
            dense_half2(op_, cg0, 0)
            dense_half2(op_, cg1, 1)
            es.close()

    nc.compile()
    return nc


_CACHE = {}


def _get_nc():
    if "nc" not in _CACHE:
        _CACHE["nc"] = build()
    return _CACHE["nc"]


def _host_prep(hidden_states, rope_cache, w_qkv, b_qkv, w_dense):
    """Build the 8 per-core input maps."""
    import ml_dtypes

    x = np.ascontiguousarray(hidden_states.reshape(TOK, H))
    xt = np.ascontiguousarray(x.T)
    # contiguous DMA tile layouts: a[k, p, t] with H index = k*128 + p
    a = xt.reshape(32, 128, TOK)
    # xt16 tile (t, gg): [128, 8, 512] -> flat row [16, 128, 4096]
    xt16 = np.empty((16, 128, 4096), np.float16)
    for t in range(4):
        for gg in range(4):
            blk = a[gg * 8:(gg + 1) * 8, :, t * 512:(t + 1) * 512]
            xt16[t * 4 + gg] = blk.transpose(1, 0, 2).reshape(128, 4096)
    # x8 tile t: [128, 32, 512] -> [4, 128, 16384]
    a8 = (a * X8S).astype(ml_dtypes.float8_e4m3)
    xt8 = np.empty((4, 128, 16384), ml_dtypes.float8_e4m3)
    for t in range(4):
        xt8[t] = a8[:, :, t * 512:(t + 1) * 512].transpose(
            1, 0, 2).reshape(128, 16384)

    # rope coefficient planes [64, TOK], token index j = b*S + s
    c0 = np.transpose(rope_cache[:, :, :, 0], (2, 1, 0)).reshape(ROT // 2, TOK)
    c1 = np.transpose(rope_cache[:, :, :, 1], (2, 1, 0)).reshape(ROT // 2, TOK)
    ra = np.repeat(c0, 2, axis=0).astype(np.float32)
    rb = np.repeat(c1, 2, axis=0).astype(np.float32)
    rb[0::2] *= -1.0
    rq = np.ascontiguousarray(np.vstack([ra, rb]))

    perm = np.zeros((ROT, ROT), np.float32)
    for k in range(ROT):
        perm[k, k ^ 1] = 1.0
    cc = np.zeros((128, 448), np.float32)
    cc[:, 0:128] = 1.0                                  # ones
    cc[:, 128:256] = np.triu(np.ones((128, 128)))       # tri[k,q]=1 iff q>=k
    cc[:, 256:384] = np.eye(128)                        # ident
    cc[0:64, 384:448] = perm
    cc = cc.astype(np.float16)

    in_maps = []
    for c in range(N_CORES):
        g = c // (N_CORES // G)     # KV group
        r = c % (N_CORES // G)      # rank within KV group
        oi = c % 4                  # dense output-quarter
        kc0 = NH * HD + g * HD + 32 * r          # K col slice start
        vc0 = NH * HD + G * HD + g * HD + 32 * r  # V col slice start
        wkv_c = np.concatenate([
            w_qkv[:, kc0:kc0 + 32],
            w_qkv[:, vc0:vc0 + 32],
        ], axis=1).astype(np.float16)
        # [128, 32, 64] -> flat [128, 2048]
        wkv_c = np.ascontiguousarray(
            wkv_c.reshape(32, 128, 64).transpose(1, 0, 2).reshape(128, 2048))
        # full group K[128]|V[128] for the local batch-0 KV path
        kf0 = NH * HD + g * HD
        vf0 = NH * HD + G * HD + g * HD
        wkvf_c = np.concatenate([
            w_qkv[:, kf0:kf0 + HD],
            w_qkv[:, vf0:vf0 + HD],
        ], axis=1).astype(np.float16)
        wkvf_c = np.ascontiguousarray(
            wkvf_c.reshape(32, 128, 256).transpose(1, 0, 2).reshape(128, 8192))
        wq8_c = (w_qkv[:, c * DPC:(c + 1) * DPC] * W8S).astype(
            ml_dtypes.float8_e4m3)
        # [128, 32, 512] -> flat [128, 16384]
        wq8_c = np.ascontiguousarray(
            wq8_c.reshape(32, 128, DPC).transpose(1, 0, 2).reshape(128, 16384))
        wdc = w_dense[:, oi * ODPC:(oi + 1) * ODPC].astype(np.float16)
        # chunk g: [128, 8, 1024] -> [4, 128, 8192]
        wdc = np.ascontiguousarray(
            wdc.reshape(4, 8, 128, ODPC).transpose(0, 2, 1, 3).reshape(
                4, 128, 8192))
        bq_c = np.zeros((128, 7), np.float32)
        bq_c[:, 0:4] = b_qkv[c * DPC:(c + 1) * DPC].reshape(4, 128).T
        bq_c[0:32, 4] = b_qkv[kc0:kc0 + 32]
        bq_c[32:64, 4] = b_qkv[vc0:vc0 + 32]
        bq_c[:, 5] = b_qkv[kf0:kf0 + HD]
        bq_c[:, 6] = b_qkv[vf0:vf0 + HD]
        if r < 2:
            rak = ra[32 * r:32 * r + 32]
            rbk = rb[32 * r:32 * r + 32]
        else:  # pass-dims: rope is identity
            rak = np.ones((32, TOK), np.float32)
            rbk = np.zeros((32, TOK), np.float32)
        in_maps.append({
            "xt": xt16,
            "x8": xt8,
            "wkv": wkv_c,
            "wkvf": wkvf_c,
            "wq8": wq8_c,
            "bqkv": np.ascontiguousarray(bq_c),
            "consts": cc,
            "ropeQ": rq.astype(np.float16),
            "ropeK": np.ascontiguousarray(
                np.vstack([rak, rbk])).astype(np.float16),
            "wd": wdc,
        })
    return in_maps


def kernel(hidden_states, rope_cache, w_qkv, b_qkv, w_dense,
           _trace=False, _trace_cores=None):
    nc = _get_nc()
    in_maps = _host_prep(np.asarray(hidden_states), np.asarray(rope_cache),
                         np.asarray(w_qkv), np.asarray(b_qkv),
                         np.asarray(w_dense))
    res = run_bass_kernel_spmd(nc, in_maps, core_ids=list(range(N_CORES)),
                               trace=_trace, trace_cores=_trace_cores)
    _CACHE["last_result"] = res
    full = np.empty((TOK, H), np.float32)
    for c in range(N_CORES):
        ti, oi = c // 4, c % 4
        o = res.results[c]["out"]                 # [1024 od, 1024 tok]
        for b in range(B):
            full[b * S + ti * 512:b * S + (ti + 1) * 512,
                 oi * ODPC:(oi + 1) * ODPC] = o[:, b * 512:(b + 1) * 512].T
    return full.reshape(B, S, H)


# revision 33
# speedup vs baseline: 1.0588x; 1.0588x over previous
"""ChatGLM2 attention block on 8 Trainium2 NeuronCores (Bass/Tile).

Sharding: tensor-parallel across heads. Core c owns Q heads 4c..4c+3
(512 dims, projected in fp8 DoubleRow at 2x PE rate — sim-verified to
leave the output metric unchanged since Q/K errors wash out in softmax);
K/V projection (64-col [K32|V32] rank slice, fp16 for V precision) is
rebuilt per batch with an intra-group AllGather. Dense is sharded 2x4
(token-half x output-quarter) behind a per-batch AllToAll.

Schedule: batch-0 K/V is computed locally in full (each core does its
group's 256 K+V dims) so attention(0) needs no collective and the first
collective (~70us startup-barrier floor) is batch-1's small AllGather,
issued before AllToAll(0) on the serial CC stream. KV blocks run before
Q blocks (fp8 x/w stream in behind the fp16 one), attention(0) sits
between Q blocks 1 and 2 so AllToAll(0) flies during Q blocks 2-3, and
AllToAll(1) rides behind dense(batch 0). All x/w DMAs use host-side
contiguous tile layouts split across the sync and scalar queues. The PE
is GPIO power-throttled to ~81% duty with 8 cores active (~1.95 cols/ns
fp16 sustained), which sets the compute floor.
"""

import math
import sys
import types

import numpy as np

# ---------------------------------------------------------------- constants
B, S, H = 2, 1024, 4096
NH, G, HD = 32, 2, 128
ROT = 64
N_CORES = 8
TOK = B * S                      # 2048
HPC = NH // N_CORES              # 4 Q heads per core
DPC = HPC * HD                   # 512 Q dims per core
TB = 4                           # token blocks of 512
QB = 2                           # q blocks of 512 per batch
ODPC = H // 4                    # 1024 output dims per core (dense quarter)
SCALE = 1.0 / math.sqrt(HD)
X8S = 32.0                       # fp8 scale for hidden states
W8S = 32.0                       # fp8 scale for Q weights
QDESC = 1.0 / (X8S * W8S)


def _install_ntff_hook():
    if "antenv.axon_hooks" in sys.modules:
        return
    import antenv  # noqa: F401

    mod = types.ModuleType("antenv.axon_hooks")
    mod._hook = None
    mod.set_axon_ntff_profile_hook = lambda h: setattr(mod, "_hook", h)
    mod.get_axon_ntff_profile_hook = lambda: mod._hook
    sys.modules["antenv.axon_hooks"] = mod
    try:
        from trn_agent_boot.trn_boot import _ntff_profile_via_ctypes

        mod._hook = _ntff_profile_via_ctypes("/opt/axon/libaxon_pjrt.so")
    except Exception:
        pass


_install_ntff_hook()

import concourse.bass as bass  # noqa: E402
import concourse.mybir as mybir  # noqa: E402
import concourse.tile as tile  # noqa: E402
from concourse import bacc  # noqa: E402
from concourse.bass_utils import run_bass_kernel_spmd  # noqa: E402

F32 = mybir.dt.float32
F16 = mybir.dt.float16
F8 = mybir.dt.float8e4
AF = mybir.ActivationFunctionType
ALU = mybir.AluOpType
PM = mybir.MatmulPerfMode


# ---------------------------------------------------------------- build
def build(trace_label="k"):
    nc = bacc.Bacc("TRN2", target_bir_lowering=False, debug=False,
                   num_devices=N_CORES)

    xt_d = nc.dram_tensor("xt", [16, 128, 4096], F16,
                          kind="ExternalInput").ap()
    x8_d = nc.dram_tensor("x8", [4, 128, 16384], F8,
                          kind="ExternalInput").ap()
    wkv_d = nc.dram_tensor("wkv", [128, 2048], F16,
                           kind="ExternalInput").ap()
    wkvf_d = nc.dram_tensor("wkvf", [128, 8192], F16,
                            kind="ExternalInput").ap()
    wq8_d = nc.dram_tensor("wq8", [128, 16384], F8,
                           kind="ExternalInput").ap()
    bq_d = nc.dram_tensor("bqkv", [128, 7], F32, kind="ExternalInput").ap()
    # packed f16 consts: [ ones(128) | tri(128) | ident(128) | perm(64c) ]
    cc_d = nc.dram_tensor("consts", [128, 448], F16, kind="ExternalInput").ap()
    rq_d = nc.dram_tensor("ropeQ", [128, TOK], F16, kind="ExternalInput").ap()
    rk_d = nc.dram_tensor("ropeK", [64, TOK], F16, kind="ExternalInput").ap()
    wd_d = nc.dram_tensor("wd", [4, 128, 8192], F16,
                          kind="ExternalInput").ap()
    out_d = nc.dram_tensor("out", [ODPC, 1024], F32, kind="ExternalOutput").ap()

    from contextlib import ExitStack

    with tile.TileContext(nc) as tc:
        with tc.tile_pool(name="consts", bufs=1) as cp, \
             tc.tile_pool(name="dram", bufs=1, space="DRAM") as dp:
            # ---- small constants (alive whole kernel)
            bias_sb = cp.tile([128, 7], F32, tag="bias")
            cc_sb = cp.tile([128, 448], F16, tag="consts")
            nc.sync.dma_start(bias_sb[:], bq_d[:])
            nc.sync.dma_start(cc_sb[:], cc_d[:])
            oc_sb = cc_sb[:, 0:128]
            tri_sb = cc_sb[:, 128:256]
            id_sb = cc_sb[:, 256:384]
            perm_sb = cc_sb[0:64, 384:448]
            # rope planes for all 4 blocks (loads emitted after x16(0))
            rqall = cp.tile([128, TOK], F16, tag="ropeQ")
            rkall = cp.tile([64, TOK], F16, tag="ropeK")

            def load_rope():
                nc.sync.dma_start(rqall[:], rq_d[:])
                nc.sync.dma_start(rkall[:], rk_d[:])

            # DRAM staging for collectives
            kv_loc = [dp.tile([64, 1024], F16, tag=f"kvl{b}",
                              name=f"kvl{b}") for b in range(B)]
            kvg = [dp.tile([4, 64, 1024], F16, tag=f"kvg{b}",
                           name=f"kvg{b}") for b in range(B)]
            a2a_in = [dp.tile([8, 512, 512], F16, tag=f"a2i{b}",
                              name=f"a2i{b}") for b in range(B)]
            a2a_out = [dp.tile([8, 512, 512], F16, tag=f"a2o{b}",
                               name=f"a2o{b}") for b in range(B)]
            a2ah_in = [dp.tile([8, 256, 512], F16, tag=f"a2hi{h_}",
                               name=f"a2hi{h_}") for h_ in range(2)]
            a2ah_out = [dp.tile([8, 256, 512], F16, tag=f"a2ho{h_}",
                                name=f"a2ho{h_}") for h_ in range(2)]

            es = ExitStack()
            pp = es.enter_context(
                tc.tile_pool(name="ps_main", bufs=8, space="PSUM"))
            kp = es.enter_context(tc.tile_pool(name="kvp", bufs=1))
            qtp = es.enter_context(tc.tile_pool(name="qtp", bufs=1))
            xcp = es.enter_context(tc.tile_pool(name="ctxp", bufs=5))
            ep = es.enter_context(tc.tile_pool(name="exp", bufs=8))
            sp = es.enter_context(tc.tile_pool(name="att_small", bufs=2))
            es2 = ExitStack()
            wp = es2.enter_context(tc.tile_pool(name="wq", bufs=1))
            xp = es2.enter_context(tc.tile_pool(name="xt", bufs=8))
            x8p = es2.enter_context(tc.tile_pool(name="x8", bufs=2))
            rp = es2.enter_context(tc.tile_pool(name="rope_tmp", bufs=2))
            kqp = es2.enter_context(tc.tile_pool(name="kq", bufs=2))

            # per-batch K / V(transposed) tiles
            ktile = [kp.tile([128, 1024], F16, tag=f"k{b}", name=f"k{b}")
                     for b in range(B)]
            vtile = [kp.tile([128, 1024], F16, tag=f"vt{b}", name=f"vt{b}")
                     for b in range(B)]
            vn = [kp.tile([128, 1024], F16, tag=f"vn{b}", name=f"vn{b}")
                  for b in range(B)]
            qtl = {}

            # ---- weights (scalar queue: parallel to the x16 sync stream)
            wkvf_sb = wp.tile([128, 32, 256], F16, tag="wkvf", name="wkvf")
            nc.scalar.dma_start(wkvf_sb[:].rearrange("p a b -> p (a b)"),
                                wkvf_d[:])
            wkv_sb = wp.tile([128, 32, 64], F16, tag="wkv", name="wkv")
            nc.scalar.dma_start(wkv_sb[:].rearrange("p a b -> p (a b)"),
                                wkv_d[:])
            wq8_sb = wp.tile([128, 32, DPC], F8, tag="wq8", name="wq8")

            def load_wq8():
                nc.scalar.dma_start(wq8_sb[:].rearrange("p a b -> p (a b)"),
                                    wq8_d[:])
            xg16_pre = {}
            xg8_pre = {}

            def prefetch_x16(t):
                for gg in range(4):
                    xg = xp.tile([128, 8, 512], F16, tag="xtblk")
                    nc.sync.dma_start(xg[:].rearrange("p a b -> p (a b)"),
                                      xt_d[t * 4 + gg])
                    xg16_pre[(t, gg)] = xg

            def prefetch_x8(t):
                x8t = x8p.tile([128, 32, 512], F8, tag="x8blk")
                nc.scalar.dma_start(x8t[:].rearrange("p a b -> p (a b)"),
                                    x8_d[t])
                xg8_pre[t] = x8t

            def rope64(dst, qab):
                """In-place ChatGLM2 rotary on rows 0:64 of a [128,512]
                f16 tile (rot dims 0:64, pass dims 64:128)."""
                sw = pp.tile([128, 512], F32, tag="bank", name="swps")
                nc.tensor.matmul(sw[0:ROT, :], perm_sb[:],
                                 dst[0:ROT, :], start=True, stop=True)
                t1 = rp.tile([ROT, 512], F32, tag="t1")
                nc.vector.tensor_mul(t1[:], dst[0:ROT, :], qab[0:64, :])
                t2 = rp.tile([ROT, 512], F32, tag="t2")
                nc.vector.tensor_mul(t2[:], sw[0:ROT, :], qab[64:128, :])
                nc.vector.tensor_add(dst[0:ROT, :], t1[:], t2[:])

            def kv_block_full(t):
                """Batch-0 blocks: full group K[128]+V[128] computed locally
                (no AllGather on the critical path for attention(0))."""
                half = t % 2
                xgs = [xg16_pre.pop((t, gg)) for gg in range(4)]
                kps = pp.tile([128, 512], F32, tag="bank", name="kfps")
                vps = pp.tile([128, 512], F32, tag="bank", name="vfps")
                for gg in range(4):
                    for kk in range(8):
                        k = gg * 8 + kk
                        nc.tensor.matmul(kps[:], wkvf_sb[:, k, 0:128],
                                         xgs[gg][:, kk, :],
                                         start=(k == 0), stop=(k == 31))
                ksl = ktile[0][:, half * 512:(half + 1) * 512]
                nc.scalar.activation(ksl, kps[:], AF.Identity,
                                     bias=bias_sb[:, 5:6])
                for gg in range(4):
                    for kk in range(8):
                        k = gg * 8 + kk
                        nc.tensor.matmul(vps[:], wkvf_sb[:, k, 128:256],
                                         xgs[gg][:, kk, :],
                                         start=(k == 0), stop=(k == 31))
                nc.scalar.activation(vtile[0][:, half * 512:(half + 1) * 512],
                                     vps[:], AF.Identity,
                                     bias=bias_sb[:, 6:7])
                rope64(ksl, rqall[:, t * 512:(t + 1) * 512])

            def kv_block_slice(t):
                """Batch-1 blocks: 64-col [K32|V32] rank slice -> kv_loc."""
                b, half = t // 2, t % 2
                xgs = [xg16_pre.pop((t, gg)) for gg in range(4)]
                ps_kv = pp.tile([128, 512], F32, tag="bank", name="kvps")
                for gg in range(4):
                    for kk in range(8):
                        k = gg * 8 + kk
                        nc.tensor.matmul(
                            ps_kv[0:64, :],
                            wkv_sb[:, k, :],
                            xgs[gg][:, kk, :],
                            start=(k == 0), stop=(k == 31),
                        )
                kq = kqp.tile([64, 512], F16, tag="kvtile", name=f"kv{t}")
                nc.scalar.activation(kq[:], ps_kv[0:64, :], AF.Identity,
                                     bias=bias_sb[0:64, 4:5])
                kab = rkall[:, t * 512:(t + 1) * 512]
                # K slice rope (identity planes on ranks holding pass-dims)
                swk = pp.tile([128, 512], F32, tag="bank", name="swkps")
                nc.tensor.matmul(swk[0:32, :], perm_sb[0:32, 0:32],
                                 kq[0:32, :], start=True, stop=True)
                t1 = rp.tile([32, 512], F32, tag="t1k", bufs=1)
                nc.vector.tensor_mul(t1[:], kq[0:32, :], kab[0:32, :])
                t2 = rp.tile([32, 512], F32, tag="t2k", bufs=1)
                nc.vector.tensor_mul(t2[:], swk[0:32, :], kab[32:64, :])
                nc.vector.tensor_add(kq[0:32, :], t1[:], t2[:])
                nc.sync.dma_start(
                    kv_loc[b][:, half * 512:(half + 1) * 512], kq[:])

            def q_block(t):
                """Q projection (fp8 DoubleRow, 4 psum banks) + bias + RoPE."""
                x8t = xg8_pre.pop(t)
                ps = [pp.tile([128, 512], F32, tag="bank",
                              name=f"qps{d}") for d in range(HPC)]
                for kp2 in range(16):
                    for d in range(HPC):
                        nc.tensor.matmul(
                            ps[d][:],
                            wq8_sb[:, 2 * kp2:2 * kp2 + 2,
                                   d * 128:(d + 1) * 128],
                            x8t[:, 2 * kp2:2 * kp2 + 2, :],
                            start=(kp2 == 0), stop=(kp2 == 15),
                            perf_mode=PM.DoubleRow,
                        )
                for h in range(HPC):
                    qt = qtp.tile([128, 512], F16,
                                  tag=f"q{h}_{t}", name=f"q{h}_{t}")
                    qtl[(h, t)] = qt
                    nc.scalar.activation(qt[:], ps[h][:], AF.Identity,
                                         bias=bias_sb[:, h:h + 1],
                                         scale=QDESC)
                for e in range(HPC):
                    rope64(qtl[(e, t)], rqall[:, t * 512:(t + 1) * 512])

            def kv_allgather(b):
                nc.gpsimd.collective_compute(
                    "AllGather", ALU.bypass,
                    replica_groups=[[0, 1, 2, 3], [4, 5, 6, 7]],
                    ins=[kv_loc[b][:].opt()],
                    outs=[kvg[b][:].opt()])

            def assemble_loads(b):
                """ktile/vtile from the gathered per-rank 64-col slices."""
                for r in range(4):
                    nc.gpsimd.dma_start(ktile[b][32 * r:32 * r + 32, :],
                                        kvg[b][r, 0:32, :])
                    nc.gpsimd.dma_start(vtile[b][32 * r:32 * r + 32, :],
                                        kvg[b][r, 32:64, :])

            def build_vn(b):
                for jj in range(2):
                    tp = pp.tile([128, 512], F16, tag="bank", name="vtrps")
                    for j in range(4):
                        nc.tensor.transpose(
                            tp[:, j * 128:(j + 1) * 128],
                            vtile[b][:, (jj * 4 + j) * 128:
                                     (jj * 4 + j + 1) * 128],
                            id_sb[:])
                    nc.scalar.copy(vn[b][:, jj * 512:(jj + 1) * 512], tp[:])

            def attn_batch(b):
                """Software-pipelined attention for one batch: flattened
                (qb, h, kt) stream with lookahead-2 sc -> rs/av."""
                units = [(qb, h) for qb in range(QB) for h in range(HPC)]
                tasks = []
                for u, (qb, h) in enumerate(units):
                    for kt in range(4 * (qb + 1)):
                        tasks.append((u, kt))
                n_kt = {u: 4 * (units[u][0] + 1) for u in range(len(units))}
                rs_ps, ctx_ps = {}, {}

                def emit_sc(u, kt):
                    qb, h = units[u]
                    tb = b * QB + qb
                    off = max(0, (kt - qb * 4) * 128)
                    N = 512 - off
                    sc = pp.tile([128, 512], F32, tag="bank", name="scps")
                    nc.tensor.matmul(sc[:, 0:N],
                                     ktile[b][:, kt * 128:(kt + 1) * 128],
                                     qtl[(h, tb)][:, off:512],
                                     start=True, stop=True)
                    e = ep.tile([128, 512], F16, tag="exp")
                    nc.scalar.activation(e[:, 0:N], sc[:, 0:N],
                                         AF.Exp, scale=SCALE)
                    if kt >= qb * 4:  # diagonal straddle: first 128 cols
                        nc.vector.tensor_mul(e[:, 0:128], e[:, 0:128],
                                             tri_sb[:])
                    return (e, off, N)

                def emit_rsav(u, kt, e, off, N):
                    qb, h = units[u]
                    if kt == 0:
                        rs_ps[u] = pp.tile([128, 512], F32, tag="bank",
                                           name="rsps")
                        ctx_ps[u] = pp.tile([128, 512], F32, tag="bank",
                                            name="ctxps")
                    first, last = kt == 0, kt == n_kt[u] - 1
                    nc.tensor.matmul(rs_ps[u][:, off:512], oc_sb[:],
                                     e[:, 0:N], start=first, stop=last)
                    nc.tensor.matmul(ctx_ps[u][:, off:512],
                                     vn[b][:, kt * 128:(kt + 1) * 128],
                                     e[:, 0:N], start=first, stop=last)
                    if last:
                        rcp = sp.tile([128, 512], F32, tag="rcp")
                        nc.vector.reciprocal_approx_fast(
                            out=rcp[:], in_=rs_ps[u][:])
                        ctxt = xcp.tile([128, 512], F16, tag="ctx")
                        nc.vector.tensor_mul(ctxt[:], ctx_ps[u][:], rcp[:])
                        # one DMA writes all 4 dup blocks: src broadcasts
                        # via a stride-0 dim after the partition dim
                        cap = ctxt[:]
                        bsrc = bass.AP(cap.tensor, cap.offset,
                                       [cap.ap[0], (0, 4), cap.ap[1]])
                        dst = a2a_in[b].rearrange("j p t -> p j t")[
                            h * 128:(h + 1) * 128, qb * 4:qb * 4 + 4, :]
                        nc.gpsimd.dma_start(dst, bsrc)

                pend = {}
                for i, (u, kt) in enumerate(tasks):
                    pend[i] = (u, kt) + emit_sc(u, kt)
                    if i - 4 >= 0:
                        emit_rsav(*pend.pop(i - 4))
                for j in sorted(pend):
                    emit_rsav(*pend.pop(j))

            def a2a(b):
                nc.gpsimd.collective_compute(
                    "AllToAll", ALU.bypass,
                    replica_groups=[list(range(N_CORES))],
                    ins=[a2a_in[b][:].opt()],
                    outs=[a2a_out[b][:].opt()])

            def a2a_half(h_):
                nc.gpsimd.collective_compute(
                    "AllToAll", ALU.bypass,
                    replica_groups=[list(range(N_CORES))],
                    ins=[a2ah_in[h_][:].opt()],
                    outs=[a2ah_out[h_][:].opt()])

            wd_sb = []

            def load_wd_all(wdp):
                for g in range(4):
                    wg = wdp.tile([128, 8, ODPC], F16, tag=f"wd{g}")
                    nc.sync.dma_start(wg[:].rearrange("p a b -> p (a b)"),
                                      wd_d[g])
                    wd_sb.append(wg)

            def load_cg_chunks(cgp, half):
                """Per-group cg tiles: dense can start on chunk 0 ~3us
                after the AllToAll lands instead of waiting a 4MB load."""
                cgs = a2a_out[half].rearrange("s (sub p) t -> p (s sub) t",
                                              sub=4)
                out = []
                for g in range(4):
                    cgt = cgp.tile([128, 8, 512], F16, tag=f"cg{half}{g}")
                    nc.sync.dma_start(cgt[:], cgs[:, g * 8:(g + 1) * 8, :])
                    out.append(cgt)
                return out

            def dense_half2(op_, cgch, half):
                ps = [pp.tile([128, 512], F32, tag="bank",
                              name=f"dps{half}_{odb}") for odb in range(8)]
                for g in range(4):
                    for odb in range(8):
                        for kk in range(8):
                            nc.tensor.matmul(
                                ps[odb][:],
                                wd_sb[g][:, kk, odb * 128:(odb + 1) * 128],
                                cgch[g][:, kk, :],
                                start=(g == 0 and kk == 0),
                                stop=(g == 3 and kk == 7))
                for odb in range(8):
                    o = op_.tile([128, 512], F32, tag="osb")
                    nc.scalar.copy(o[:], ps[odb][:])
                    nc.scalar.dma_start(
                        out_d[odb * 128:(odb + 1) * 128,
                              half * 512:(half + 1) * 512], o[:])

            # ---------------- schedule ----------------
            prefetch_x16(0)
            prefetch_x16(1)
            load_rope()
            prefetch_x8(0)
            prefetch_x8(1)
            load_wq8()
            kv_block_full(0)
            prefetch_x16(2)
            kv_block_full(1)
            prefetch_x16(3)
            build_vn(0)
            kv_block_slice(2)
            kv_block_slice(3)
            kv_allgather(1)
            assemble_loads(1)
            q_block(0)
            prefetch_x8(2)
            q_block(1)
            prefetch_x8(3)
            attn_batch(0)
            a2a(0)
            q_block(2)
            q_block(3)
            build_vn(1)
            es2.close()

            # dense-side pools come alive only after the proj pools free
            wdp = es.enter_context(
                tc.tile_pool(name="wd", bufs=1, side="right"))
            cgp = es.enter_context(
                tc.tile_pool(name="cg", bufs=1, side="right"))
            op_ = es.enter_context(
                tc.tile_pool(name="dout", bufs=4, side="right"))
            load_wd_all(wdp)
            cg0 = load_cg_chunks(cgp, 0)
            attn_batch(1)
            a2a(1)
            cg1 = load_cg_chunks(cgp, 1)
            dense_half2(op_, cg0, 0)
            dense_half2(op_, cg1, 1)
            es.close()

    nc.compile()
    return nc


_CACHE = {}


def _get_nc():
    if "nc" not in _CACHE:
        _CACHE["nc"] = build()
    return _CACHE["nc"]


def _host_prep(hidden_states, rope_cache, w_qkv, b_qkv, w_dense):
    """Build the 8 per-core input maps."""
    import ml_dtypes

    x = np.ascontiguousarray(hidden_states.reshape(TOK, H))
    xt = np.ascontiguousarray(x.T)
    # contiguous DMA tile layouts: a[k, p, t] with H index = k*128 + p
    a = xt.reshape(32, 128, TOK)
    # xt16 tile (t, gg): [128, 8, 512] -> flat row [16, 128, 4096]
    xt16 = np.empty((16, 128, 4096), np.float16)
    for t in range(4):
        for gg in range(4):
            blk = a[gg * 8:(gg + 1) * 8, :, t * 512:(t + 1) * 512]
            xt16[t * 4 + gg] = blk.transpose(1, 0, 2).reshape(128, 4096)
    # x8 tile t: [128, 32, 512] -> [4, 128, 16384]
    a8 = (a * X8S).astype(ml_dtypes.float8_e4m3)
    xt8 = np.empty((4, 128, 16384), ml_dtypes.float8_e4m3)
    for t in range(4):
        xt8[t] = a8[:, :, t * 512:(t + 1) * 512].transpose(
            1, 0, 2).reshape(128, 16384)

    # rope coefficient planes [64, TOK], token index j = b*S + s
    c0 = np.transpose(rope_cache[:, :, :, 0], (2, 1, 0)).reshape(ROT // 2, TOK)
    c1 = np.transpose(rope_cache[:, :, :, 1], (2, 1, 0)).reshape(ROT // 2, TOK)
    ra = np.repeat(c0, 2, axis=0).astype(np.float32)
    rb = np.repeat(c1, 2, axis=0).astype(np.float32)
    rb[0::2] *= -1.0
    rq = np.ascontiguousarray(np.vstack([ra, rb]))

    perm = np.zeros((ROT, ROT), np.float32)
    for k in range(ROT):
        perm[k, k ^ 1] = 1.0
    cc = np.zeros((128, 448), np.float32)
    cc[:, 0:128] = 1.0                                  # ones
    cc[:, 128:256] = np.triu(np.ones((128, 128)))       # tri[k,q]=1 iff q>=k
    cc[:, 256:384] = np.eye(128)                        # ident
    cc[0:64, 384:448] = perm
    cc = cc.astype(np.float16)

    in_maps = []
    for c in range(N_CORES):
        g = c // (N_CORES // G)     # KV group
        r = c % (N_CORES // G)      # rank within KV group
        oi = c % 4                  # dense output-quarter
        kc0 = NH * HD + g * HD + 32 * r          # K col slice start
        vc0 = NH * HD + G * HD + g * HD + 32 * r  # V col slice start
        wkv_c = np.concatenate([
            w_qkv[:, kc0:kc0 + 32],
            w_qkv[:, vc0:vc0 + 32],
        ], axis=1).astype(np.float16)
        # [128, 32, 64] -> flat [128, 2048]
        wkv_c = np.ascontiguousarray(
            wkv_c.reshape(32, 128, 64).transpose(1, 0, 2).reshape(128, 2048))
        # full group K[128]|V[128] for the local batch-0 KV path
        kf0 = NH * HD + g * HD
        vf0 = NH * HD + G * HD + g * HD
        wkvf_c = np.concatenate([
            w_qkv[:, kf0:kf0 + HD],
            w_qkv[:, vf0:vf0 + HD],
        ], axis=1).astype(np.float16)
        wkvf_c = np.ascontiguousarray(
            wkvf_c.reshape(32, 128, 256).transpose(1, 0, 2).reshape(128, 8192))
        wq8_c = (w_qkv[:, c * DPC:(c + 1) * DPC] * W8S).astype(
            ml_dtypes.float8_e4m3)
        # [128, 32, 512] -> flat [128, 16384]
        wq8_c = np.ascontiguousarray(
            wq8_c.reshape(32, 128, DPC).transpose(1, 0, 2).reshape(128, 16384))
        wdc = w_dense[:, oi * ODPC:(oi + 1) * ODPC].astype(np.float16)
        # chunk g: [128, 8, 1024] -> [4, 128, 8192]
        wdc = np.ascontiguousarray(
            wdc.reshape(4, 8, 128, ODPC).transpose(0, 2, 1, 3).reshape(
                4, 128, 8192))
        bq_c = np.zeros((128, 7), np.float32)
        bq_c[:, 0:4] = b_qkv[c * DPC:(c + 1) * DPC].reshape(4, 128).T
        bq_c[0:32, 4] = b_qkv[kc0:kc0 + 32]
        bq_c[32:64, 4] = b_qkv[vc0:vc0 + 32]
        bq_c[:, 5] = b_qkv[kf0:kf0 + HD]
        bq_c[:, 6] = b_qkv[vf0:vf0 + HD]
        if r < 2:
            rak = ra[32 * r:32 * r + 32]
            rbk = rb[32 * r:32 * r + 32]
        else:  # pass-dims: rope is identity
            rak = np.ones((32, TOK), np.float32)
            rbk = np.zeros((32, TOK), np.float32)
        in_maps.append({
            "xt": xt16,
            "x8": xt8,
            "wkv": wkv_c,
            "wkvf": wkvf_c,
            "wq8": wq8_c,
            "bqkv": np.ascontiguousarray(bq_c),
            "consts": cc,
            "ropeQ": rq.astype(np.float16),
            "ropeK": np.ascontiguousarray(
                np.vstack([rak, rbk])).astype(np.float16),
            "wd": wdc,
        })
    return in_maps


def kernel(hidden_states, rope_cache, w_qkv, b_qkv, w_dense,
           _trace=False, _trace_cores=None):
    nc = _get_nc()
    in_maps = _host_prep(np.asarray(hidden_states), np.asarray(rope_cache),
                         np.asarray(w_qkv), np.asarray(b_qkv),
                         np.asarray(w_dense))
    res = run_bass_kernel_spmd(nc, in_maps, core_ids=list(range(N_CORES)),
                               trace=_trace, trace_cores=_trace_cores)
    _CACHE["last_result"] = res
    full = np.empty((TOK, H), np.float32)
    for c in range(N_CORES):
        ti, oi = c // 4, c % 4
        o = res.results[c]["out"]                 # [1024 od, 1024 tok]
        for b in range(B):
            full[b * S + ti * 512:b * S + (ti + 1) * 512,
                 oi * ODPC:(oi + 1) * ODPC] = o[:, b * 512:(b + 1) * 512].T
    return full.reshape(B, S, H)
